# revision 6
# baseline (speedup 1.0000x reference)
"""Trainium2 Bass kernel for nn_GN_89266600280080.

Computes, for output[B,O], input[B,D], weights[O]:
    dl_dW = (1/B) * (output * weights)^T @ input        # [O, D]
    gw    = sqrt(sum(dl_dW^2, axis=1))                  # [O]

Strategy (8 NeuronCores, data-parallel over batch):
  - host: shard output/input on batch, pre-pack per-core slices into
    [128, n_blk, *] partition-major layout and quantize to fp8e4m3
    (norm over D=1024 averages quantization noise: ~3e-3 rel err vs
    2e-2 tolerance). Weight folding deferred to the final scalar.
  - device: M_partial^T stacked = output_loc^T @ input_loc via fp8
    matmuls, 4-way column-tiled (tile_position=(0,32g)) so 4 batch
    blocks stream concurrently through disjoint 32-col strips of the
    PE array -> 4x effective matmul throughput, hidden under DMA.
  - selector matmul sums the 4 stacked [32, D] group partials.
  - collective=True: AllReduce(add) across cores + on-device norm.
    collective=False: each core emits its [O, D] partial; host sums
    across cores and finishes the (tiny) norm.
"""

import sys
import numpy as np
import ml_dtypes

for _p in ("/opt/trn_rl_repo", "/root/.axon_site/_ro/trn_rl_repo"):
    if _p not in sys.path:
        sys.path.insert(0, _p)

B, O, D = 32768, 32, 1024
N_CORES = 8
B_LOC = B // N_CORES   # 4096
P = 128                # partitions
NBLK = B_LOC // P      # 32 batch blocks of 128 rows per core
NH = 2                 # D split into NH chunks of 512 for psum banks
ND = D // NH           # 512
CH = 4                 # batch blocks per DMA chunk
NGRP = 4               # concurrent PE column groups
NP8 = ml_dtypes.float8_e4m3
COLLECTIVE = False     # False: host finishes cross-core sum + norm


def build(n_cores=N_CORES, b_total=B, collective=COLLECTIVE):
    """Build + compile the per-core Bass program. Returns the Bacc object."""
    import concourse.bacc as bacc
    import concourse.tile as tile
    import concourse.mybir as mybir

    f32 = mybir.dt.float32
    bf16 = mybir.dt.bfloat16
    f8 = mybir.dt.float8e4
    nsteps = NBLK // NGRP  # accumulation steps per column group

    nc = bacc.Bacc("TRN2", target_bir_lowering=False, debug=False,
                   num_devices=n_cores)

    # host pre-packed layouts (partition-major, contiguous per partition)
    in_d = nc.dram_tensor("input", [P, NBLK, D], f8, kind="ExternalInput")
    out_d = nc.dram_tensor("output", [P, NBLK, O], f8, kind="ExternalInput")
    sel_d = nc.dram_tensor("sel", [P, O], bf16, kind="ExternalInput")
    if collective:
        w_d = nc.dram_tensor("weights", [O], f32, kind="ExternalInput")
        gw_d = nc.dram_tensor("gw", [O], f32, kind="ExternalOutput")
    else:
        part_d = nc.dram_tensor("part", [O, D], f32, kind="ExternalOutput")

    with tile.TileContext(nc) as tc:
        with (
            tc.tile_pool(name="wout", bufs=1) as wout_pool,
            tc.tile_pool(name="rhs", bufs=3) as rhs_pool,
            tc.tile_pool(name="ps", bufs=2, space="PSUM") as psum_pool,
            tc.tile_pool(name="misc", bufs=1) as misc,
            tc.tile_pool(name="dram", bufs=1, space="DRAM") as dram_pool,
        ):
            # stationary operand: all local w_out rows, [128, 32, 32] fp8
            wout = wout_pool.tile([P, NBLK, O], f8)
            nc.sync.dma_start(wout[:], out_d.ap())

            sel_sb = misc.tile([P, O], bf16)
            nc.sync.dma_start(sel_sb[:], sel_d.ap())

            if collective:
                w_sb = misc.tile([O, 1], f32)
                nc.sync.dma_start(
                    w_sb[:], w_d.ap().rearrange("(p one) -> p one", one=1))
                # w2 = (w / B)^2 exactly on DVE
                w_n = misc.tile([O, 1], f32)
                nc.vector.tensor_scalar_mul(w_n[:], w_sb[:], 1.0 / b_total)
                w2 = misc.tile([O, 1], f32)
                nc.vector.tensor_mul(w2[:], w_n[:], w_n[:])

            # stacked partials: group g accumulates into partitions 32g..32g+31
            psum = psum_pool.tile([P, D], f32)
            for c in range(NBLK // CH):
                rhs = rhs_pool.tile([P, CH, D], f8)
                nc.sync.dma_start(rhs[:], in_d.ap()[:, c * CH:(c + 1) * CH, :])
                for j in range(CH):
                    blk = c * CH + j
                    g, t = blk % NGRP, blk // NGRP
                    for h in range(NH):
                        nc.tensor.matmul(
                            psum[O * g:O * (g + 1), h * ND:(h + 1) * ND],
                            wout[:, blk, :],
                            rhs[:, j, h * ND:(h + 1) * ND],
                            start=(t == 0),
                            stop=(t == nsteps - 1),
                            tile_position=(0, O * g),
                        )

            # sum the 4 stacked [32, D] group partials via selector matmul:
            # psum2[o, d] = sum_p sel[p, o] * stacked[p, d],  sel[p,o]=(p%32==o)
            stacked_bf = misc.tile([P, D], bf16)
            nc.vector.tensor_copy(stacked_bf[:], psum[:])
            psum2 = psum_pool.tile([O, D], f32)
            for h in range(NH):
                nc.tensor.matmul(
                    psum2[:, h * ND:(h + 1) * ND],
                    sel_sb[:],
                    stacked_bf[:, h * ND:(h + 1) * ND],
                    start=True, stop=True,
                )

            part_sb = misc.tile([O, D], f32)
            nc.vector.tensor_copy(part_sb[:], psum2[:])

            if not collective:
                nc.sync.dma_start(part_d.ap(), part_sb[:])
            else:
                part_dram = dram_pool.tile([O, D], f32)
                nc.sync.dma_start(part_dram[:], part_sb[:])

                red_dram = dram_pool.tile([O, D], f32)
                nc.gpsimd.collective_compute(
                    "AllReduce",
                    mybir.AluOpType.add,
                    replica_groups=[list(range(n_cores))],
                    ins=[part_dram.opt()],
                    outs=[red_dram.opt()],
                )

                red_sb = misc.tile([O, D], f32)
                nc.sync.dma_start(red_sb[:], red_dram[:])

                # ss[o] = sum_d red[o,d]^2  (square + reduce on DVE)
                sq = misc.tile([O, D], f32)
                nc.vector.tensor_mul(sq[:], red_sb[:], red_sb[:])
                ss = misc.tile([O, 1], f32)
                nc.vector.reduce_sum(ss[:], sq[:], axis=mybir.AxisListType.X)

                # gw = sqrt(ss * (w/B)^2)
                gw_sb = misc.tile([O, 1], f32)
                nc.scalar.activation(
                    gw_sb[:], ss[:], mybir.ActivationFunctionType.Sqrt,
                    bias=0.0, scale=w2[:])
                nc.sync.dma_start(
                    gw_d.ap().rearrange("(p one) -> p one", one=1), gw_sb[:])

    nc.compile()
    return nc


_CACHE = {}


def _get_nc(collective=COLLECTIVE):
    key = f"nc{collective}"
    if key not in _CACHE:
        _CACHE[key] = build(collective=collective)
    return _CACHE[key]


def _pack(arr, ncols):
    """[B_LOC, ncols] fp32 -> [128, NBLK, ncols] fp8, partition-major."""
    return np.ascontiguousarray(
        arr.reshape(NBLK, P, ncols).transpose(1, 0, 2)).astype(NP8)


def _sel_mat():
    p = np.arange(P)[:, None]
    o = np.arange(O)[None, :]
    return ((p % O) == o).astype(ml_dtypes.bfloat16)


def prep_in_maps(inputs):
    output = np.asarray(inputs["output"], dtype=np.float32)
    input = np.asarray(inputs["input"], dtype=np.float32)
    weights = np.asarray(inputs["weights"], dtype=np.float32)
    sel = _sel_mat()
    maps = []
    for c in range(N_CORES):
        m = {
            "output": _pack(output[c * B_LOC:(c + 1) * B_LOC], O),
            "input": _pack(input[c * B_LOC:(c + 1) * B_LOC], D),
            "sel": sel,
        }
        if COLLECTIVE:
            m["weights"] = weights
        maps.append(m)
    return maps


def kernel(output, input, weights):
    from concourse.bass_utils import run_bass_kernel_spmd

    weights = np.asarray(weights, dtype=np.float32)
    nc = _get_nc()
    in_maps = prep_in_maps(
        {"output": output, "input": input, "weights": weights})
    res = run_bass_kernel_spmd(nc, in_maps, list(range(N_CORES)))
    if COLLECTIVE:
        return np.asarray(res.results[0]["gw"], dtype=np.float32).reshape(O)
    # host finish: sum per-core [O, D] partials, then the tiny norm
    M = np.zeros((O, D), dtype=np.float64)
    for r in res.results:
        M += np.asarray(r["part"], dtype=np.float64)
    ss = (M * M).sum(axis=1)
    gw = np.sqrt(ss) * (weights.astype(np.float64) / B)
    return gw.astype(np.float32)


# revision 7
# speedup vs baseline: 1.0883x; 1.0883x over previous
"""Trainium2 Bass kernel for nn_GN_89266600280080.

Computes, for output[B,O], input[B,D], weights[O]:
    dl_dW = (1/B) * (output * weights)^T @ input        # [O, D]
    gw    = sqrt(sum(dl_dW^2, axis=1))                  # [O]

Strategy (8 NeuronCores, data-parallel over batch):
  - host: shard output/input on batch, pre-pack per-core slices into
    [128, n_blk, *] partition-major layout and quantize to fp8e4m3
    (norm over D=1024 averages quantization noise: ~3e-3 rel err vs
    2e-2 tolerance). Weight folding deferred to the final host scalar.
  - device: stacked partials = output_loc^T @ input_loc via fp8
    matmuls, 4-way column-tiled (tile_position=(0,32g)): 4 batch
    blocks stream concurrently through disjoint 32-col strips of the
    PE array. Input DMA round-robins over the 3 DMA rings (sync-HWDGE,
    scalar-HWDGE, gpsimd-SWDGE) with coalesced 4KB/partition
    descriptors to overlap ring throughputs.
  - each core emits its stacked [128, D] partial (bf16); host sums
    the 4 column groups + 8 cores and finishes the (tiny) norm.
"""

import sys
import numpy as np
import ml_dtypes

for _p in ("/opt/trn_rl_repo", "/root/.axon_site/_ro/trn_rl_repo"):
    if _p not in sys.path:
        sys.path.insert(0, _p)

B, O, D = 32768, 32, 1024
N_CORES = 8
B_LOC = B // N_CORES   # 4096
P = 128                # partitions
NBLK = B_LOC // P      # 32 batch blocks of 128 rows per core
NH = 2                 # D split into NH chunks of 512 for psum banks
ND = D // NH           # 512
CH = 4                 # batch blocks per DMA chunk
NGRP = 4               # concurrent PE column groups
NP8 = ml_dtypes.float8_e4m3


def build(n_cores=N_CORES):
    """Build + compile the per-core Bass program. Returns the Bacc object."""
    import concourse.bacc as bacc
    import concourse.tile as tile
    import concourse.mybir as mybir

    f32 = mybir.dt.float32
    bf16 = mybir.dt.bfloat16
    f8 = mybir.dt.float8e4
    nsteps = NBLK // NGRP  # accumulation steps per column group

    nc = bacc.Bacc("TRN2", target_bir_lowering=False, debug=False,
                   num_devices=n_cores)

    # host pre-packed layouts (partition-major, contiguous per partition)
    in_d = nc.dram_tensor("input", [P, NBLK, D], f8, kind="ExternalInput")
    out_d = nc.dram_tensor("output", [P, NBLK, O], f8, kind="ExternalInput")
    part_d = nc.dram_tensor("part", [P, D], bf16, kind="ExternalOutput")

    in_2d = in_d.ap().rearrange("p n d -> p (n d)")

    with tile.TileContext(nc) as tc:
        with (
            tc.tile_pool(name="wout", bufs=1) as wout_pool,
            tc.tile_pool(name="rhs", bufs=4) as rhs_pool,
            tc.tile_pool(name="ps", bufs=1, space="PSUM") as psum_pool,
            tc.tile_pool(name="misc", bufs=1) as misc,
        ):
            # stationary operand: all local w_out rows, [128, 32, 32] fp8
            wout = wout_pool.tile([P, NBLK, O], f8)
            nc.scalar.dma_start(wout[:], out_d.ap())

            dma_engines = [nc.sync, nc.scalar, nc.gpsimd]

            # stacked partials: group g accumulates into partitions 32g..32g+31
            psum = psum_pool.tile([P, D], f32)
            for c in range(NBLK // CH):
                rhs = rhs_pool.tile([P, CH, D], f8)
                # coalesced 2D AP -> 4KB contiguous per partition
                dma_engines[c % 3].dma_start(
                    rhs[:].rearrange("p n d -> p (n d)"),
                    in_2d[:, c * CH * D:(c + 1) * CH * D])
                for h in range(NH):
                    for j in range(CH):
                        blk = c * CH + j
                        g, t = blk % NGRP, blk // NGRP
                        nc.tensor.matmul(
                            psum[O * g:O * (g + 1), h * ND:(h + 1) * ND],
                            wout[:, blk, :],
                            rhs[:, j, h * ND:(h + 1) * ND],
                            start=(t == 0),
                            stop=(t == nsteps - 1),
                            tile_position=(0, O * g),
                        )

            # cast stacked psum to bf16 and ship it out; host sums the
            # 4 groups + 8 cores and finishes the norm.
            stacked_bf = misc.tile([P, D], bf16)
            nc.vector.tensor_copy(stacked_bf[:], psum[:])
            nc.sync.dma_start(part_d.ap(), stacked_bf[:])

    nc.compile()
    return nc


_CACHE = {}


def _get_nc():
    if "nc" not in _CACHE:
        _CACHE["nc"] = build()
    return _CACHE["nc"]


def _pack(arr, ncols):
    """[B_LOC, ncols] fp32 -> [128, NBLK, ncols] fp8, partition-major."""
    return np.ascontiguousarray(
        arr.reshape(NBLK, P, ncols).transpose(1, 0, 2)).astype(NP8)


def prep_in_maps(inputs):
    output = np.asarray(inputs["output"], dtype=np.float32)
    input = np.asarray(inputs["input"], dtype=np.float32)
    return [
        {
            "output": _pack(output[c * B_LOC:(c + 1) * B_LOC], O),
            "input": _pack(input[c * B_LOC:(c + 1) * B_LOC], D),
        }
        for c in range(N_CORES)
    ]


def kernel(output, input, weights):
    from concourse.bass_utils import run_bass_kernel_spmd

    weights = np.asarray(weights, dtype=np.float32)
    nc = _get_nc()
    in_maps = prep_in_maps({"output": output, "input": input})
    res = run_bass_kernel_spmd(nc, in_maps, list(range(N_CORES)))
    # host finish: sum 4 col-groups + 8 cores, then the tiny norm
    M = np.zeros((O, D), dtype=np.float64)
    for r in res.results:
        part = np.asarray(r["part"]).astype(np.float64)       # [128, D]
        M += part.reshape(NGRP, O, D).sum(axis=0)
    ss = (M * M).sum(axis=1)
    gw = np.sqrt(ss) * (weights.astype(np.float64) / B)
    return gw.astype(np.float32)
